# revision 25
# baseline (speedup 1.0000x reference)
"""Trainium2 Bass kernel for nn_CAA_Stable (stable-diffusion style spatial
self-attention over 64x64 feature maps), v2: exp-cache software pipeline.

Reference computation per batch b (C=256 channels, N=64*64=4096 positions):
    q = scale*(Wq@x + bq)  [D=16, N]   (scale folded into the exp)
    k = Wk@x + bk          [D, N]
    logits[n,m] = q[:,n].k[:,m];  w = softmax(logits, axis=m)
    y = gamma_clipped * (Wo @ ((Wv@x+bv) @ w^T) + bo) + x

v2 design (vs v1's serial phase-0 + same-strip AV at ~255us):
  * One-strip-lagged AV: strip s's QK+exp stream runs while the AV/den
    DoubleRow matmuls for strip s-1 consume exp tiles CACHED IN SBUF (expp
    pool, 32 bufs = 2 strips of lag). The PE never waits on the ACT, and --
    the real point -- the ua/den PSUM banks are idle during strip 0, so all
    production work (U^T chunks, k projection) pipelines into strip 0
    instead of running as a ~60us serial phase 0.
  * q/k projections write their NB row-band replicas directly from the PE:
    stationary wq4/wk4 = [wqT 0 wqT 0 ...] (zero-gapped band replication),
    so v1's 48 SBUF->SBUF replication DMAs (~35us of HWDGE) are gone. The
    QK matmuls contract over just the D=16 band rows (k16=1), so no
    zero-padding of band gaps is needed at all.
  * Every dma_start costs ~630ns serialized on the shared HWDGE, so DMAs
    are batched (x in 2 chunks/queue, Wv/Wo/bv/bo pair-loaded) and y writes
    go through the Pool-engine SWDGE. Biases are band-replicated by a tiny
    PE matmul against a constant selector instead of 8 separate DMAs;
    gamma broadcasts straight from DRAM.
  * dve_exp of the 16 exp groups per strip run on the DVE as the
    piecewise-linear 2^y bit trick (byte = logit*s*4*log2(e)+60 == e5m2 of
    exp(s*logit)), balancing the ACT (the v1 bottleneck at ~134us busy)
    against the otherwise underused DVE. The softmax scale rides the exp
    (ACT activation scale operand / the PWL multiplier), not the weights.
  * W2 = Wo@Wv precomputed on the PE; U = W2@x replaces the whole v
    projection; Wo@bv and clipped gamma fold into the epilogue constant.
  * softmax exp -> fp8 e5m2 (zero subnormals/saturation for this logit
    range), AV + ones-matmul denominator in e5m2 DoubleRow (256-key
    contraction per pass).
PSUM: qk 2x[128,1024] double-buffered (4 banks) + ua 2x[128,512] (2, ut
production psums in strip 0) + den/proj scratch [128,512] x2 (2).

Timing: per-rep time = slope between loop_repeat=64 and 256 hardware-loop
builds, device-resident serialized dispatch, median (see bench_slope.py);
host dispatch overhead drifts, single-dispatch measurements are noise.

Sharding: pure data-parallel over batch, one image per NeuronCore, no
collectives. kernel() takes FULL inputs, returns the FULL output.
"""

import numpy as np

B, C, HW, D = 8, 256, 4096, 16
P = 128
QS = 512              # q-strip width (one PSUM bank)
NSTRIP = HW // QS     # 8
NKC = HW // P         # 32 key chunks of 128
KGRP = 2              # key chunks per group (one qk tile)
NGRP = NKC // KGRP    # 16
SCALE = float(D) ** -0.5

_cache = {}

DEFAULTS = dict(dve_exp=4)


def _build(
    dve_exp=None,
    dve_cols=None,
    den_b=1,
    repeat=1,
    loop_repeat=0,
    k16=1,
    repl_pack=1,
    dbg=0,
):
    import ml_dtypes
    import concourse.bacc as bacc
    import concourse.mybir as mybir
    import concourse.tile as tile

    if dve_exp is None:
        dve_exp = DEFAULTS["dve_exp"]
    if dve_cols is None:
        dve_cols = DEFAULTS.get("dve_cols", 0)

    dt = mybir.dt
    AF = mybir.ActivationFunctionType
    ALU = mybir.AluOpType
    f32, bf16, f8e5 = dt.float32, dt.bfloat16, dt.float8e5
    PM = mybir.MatmulPerfMode

    nc = bacc.Bacc("TRN2", target_bir_lowering=False, debug=False, num_devices=B)

    x_d = nc.dram_tensor("x", [C, HW], f32, kind="ExternalInput")
    wq_d = nc.dram_tensor("Wq", [D, C], f32, kind="ExternalInput")
    bq_d = nc.dram_tensor("bq", [D], f32, kind="ExternalInput")
    wk_d = nc.dram_tensor("Wk", [D, C], f32, kind="ExternalInput")
    bk_d = nc.dram_tensor("bk", [D], f32, kind="ExternalInput")
    wv_d = nc.dram_tensor("Wv", [C, C], f32, kind="ExternalInput")
    bv_d = nc.dram_tensor("bv", [C], f32, kind="ExternalInput")
    wo_d = nc.dram_tensor("Wo", [C, C], f32, kind="ExternalInput")
    bo_d = nc.dram_tensor("bo", [C], f32, kind="ExternalInput")
    g_d = nc.dram_tensor("gamma", [1], f32, kind="ExternalInput")
    y_d = nc.dram_tensor("y", [C, HW], f32, kind="ExternalOutput")

    # q/k replicated across NB row bands (32-row spacing for tile_position)
    NB = 4 if repl_pack else 1
    RW = 32 * (NB - 1) + D  # written partition rows: 112 packed, 16 unpacked
    KW = D if k16 else 32   # qk contraction rows per band

    id_d = nc.inline_tensor(np.eye(P, dtype=np.float32), name="ident_c")
    ones82_d = nc.inline_tensor(
        np.ones((P, 2, P), dtype=ml_dtypes.float8_e5m2), name="ones82_c"
    )
    rep4_np = np.zeros((D, RW), dtype=np.float32)
    for r in range(NB):
        rep4_np[np.arange(D), 32 * r + np.arange(D)] = 1.0
    rep4_d = nc.inline_tensor(rep4_np, name="rep4_c")

    x3 = x_d.ap().rearrange("(a p) n -> a p n", p=P)
    y3 = y_d.ap().rearrange("(a p) n -> a p n", p=P)
    wv3 = wv_d.ap().rearrange("(a p) c -> p a c", p=P)
    wo3 = wo_d.ap().rearrange("(a p) c -> p a c", p=P)
    bv2 = bv_d.ap().rearrange("(a p) -> p a", p=P)
    bo2 = bo_d.ap().rearrange("(a p) -> p a", p=P)

    # groups (per strip) whose exp runs on the DVE instead of the ACT:
    # byte = round(logit*scale * 4*log2(e) + 60) == e5m2(2^(logit*scale*log2 e))
    # (carry from the 2-bit mantissa rounds into the exponent field).
    dve_groups = frozenset(
        g
        for g in range(NGRP)
        if ((g + 1) * dve_exp) // NGRP > (g * dve_exp) // NGRP
    )
    PLS = SCALE * 4.0 * 1.4426950408889634
    PLB = 60.0

    with tile.TileContext(nc) as tc:
        with (
            tc.tile_pool(name="const", bufs=1) as constp,
            tc.tile_pool(name="xpool", bufs=1) as xpool,
            tc.tile_pool(name="wpool", bufs=1) as wpool,
            tc.tile_pool(name="big", bufs=1) as big,
            tc.tile_pool(name="expp", bufs=2 * NGRP) as expp,
            tc.tile_pool(name="finp", bufs=2) as finp,
            tc.tile_pool(name="qkps", bufs=2, space="PSUM") as qkps,
            tc.tile_pool(name="uaps", bufs=2, space="PSUM") as uaps,
            tc.tile_pool(name="dps", bufs=2, space="PSUM") as dps,
        ):
            # exp table load with no input deps: reads its own uninit tile
            warm = constp.tile([1, 1], f32, tag="warm", name="warm")
            nc.scalar.activation(warm[:], warm[:], AF.Exp)

            ident = constp.tile([P, P], f32, tag="ident", name="ident")
            ones_82 = constp.tile([P, 2, P], f8e5, tag="ones_82", name="ones_82")
            rep4 = constp.tile([D, RW], f32, tag="rep4", name="rep4")

            # persistent replicated q (2 strip buffers) / k tiles. With k16
            # the qk matmuls only read the D valid rows of each band, so the
            # band gaps and tail rows never need zeroing.
            q_rep = [
                big.tile([P, QS], bf16, tag=f"q_rep{i}", name=f"q_rep{i}")
                for i in range(2)
            ]
            k_rep = big.tile([P, HW], bf16, tag="k_rep", name="k_rep")
            if not k16:
                for t in (*q_rep, k_rep):
                    nc.vector.memset(t[:], 0.0)

            def _emit_rep():
                # ---------- prologue: loads (SP queue, critical first)
                xs = []
                for ci in range(2):
                    t = xpool.tile([P, HW], f32, tag=f"x{ci}", name=f"x{ci}")
                    xs.append(t)
                qq = [nc.sync, nc.gpsimd]
                for ci in range(2):
                    qq[ci].dma_start(xs[ci][:, : 2 * QS], x3[ci][:, : 2 * QS])
                wq_sb = wpool.tile([D, C], f32, tag="wq", name="wq")
                nc.sync.dma_start(wq_sb[:], wq_d.ap())
                wk_sb = wpool.tile([D, C], f32, tag="wk", name="wk")
                nc.sync.dma_start(wk_sb[:], wk_d.ap())
                nc.sync.dma_start(ident[:], id_d.ap())

                with nc.allow_non_contiguous_dma(reason="tiny bias vectors"):
                    bq_sb = wpool.tile([D, 1], f32, tag="bq", name="bq")
                    nc.sync.dma_start(bq_sb[:], bq_d.ap()[:, None])
                    bk_sb = wpool.tile([D, 1], f32, tag="bk", name="bk")
                    nc.sync.dma_start(bk_sb[:], bk_d.ap()[:, None])
                nc.sync.dma_start(rep4[:], rep4_d.ap())
                # x strips 2-7 chunked so no single transfer blocks the small
                # weight loads on the serial DMA device
                for ci in range(2):
                    qq[ci].dma_start(
                        xs[ci][:, 2 * QS : 4 * QS], x3[ci][:, 2 * QS : 4 * QS]
                    )
                wv_sb = wpool.tile([P, 2, C], f32, tag="wv", name="wv")
                nc.sync.dma_start(wv_sb[:], wv3)
                wo_sb = wpool.tile([P, 2, C], f32, tag="wo", name="wo")
                nc.sync.dma_start(wo_sb[:], wo3)
                for ci in range(2):
                    qq[ci].dma_start(
                        xs[ci][:, 4 * QS : 6 * QS], x3[ci][:, 4 * QS : 6 * QS]
                    )
                bv_sb = wpool.tile([P, 2], f32, tag="bv", name="bv")
                nc.sync.dma_start(bv_sb[:], bv2)
                bo_sb = wpool.tile([P, 2], f32, tag="bo", name="bo")
                nc.sync.dma_start(bo_sb[:], bo2)
                g_rep = wpool.tile([P, 1], f32, tag="grep", name="grep")
                nc.sync.dma_start(g_rep[:], g_d.ap()[:, None].to_broadcast((P, 1)))
                for ci in range(2):
                    qq[ci].dma_start(
                        xs[ci][:, 6 * QS :], x3[ci][:, 6 * QS :]
                    )
                nc.sync.dma_start(ones_82[:], ones82_d.ap())

                xb = []
                for ci in range(2):
                    tb = xpool.tile([P, HW], bf16, tag=f"xb{ci}", name=f"xb{ci}")
                    xb.append(tb)

                def xb_copy(j):  # strip j cols -> bf16, both channel tiles
                    sl = slice(j * QS, (j + 1) * QS)
                    for ci in range(2):
                        nc.vector.tensor_copy(xb[ci][:, sl], xs[ci][:, sl])

                # transposed wq/wk replicated at the NB 32-row bands; band-
                # replicated biases via a tiny matmul against the rep4
                # selector (one DMA instead of 8).
                wq4, wk4 = [], []
                for ci in range(2):
                    t = wpool.tile([P, RW], bf16, tag=f"wq4{ci}", name=f"wq4{ci}")
                    wq4.append(t)
                    t = wpool.tile([P, RW], bf16, tag=f"wk4{ci}", name=f"wk4{ci}")
                    wk4.append(t)
                bq4 = wpool.tile([RW, 1], f32, tag="bq4", name="bq4")
                bk4 = wpool.tile([RW, 1], f32, tag="bk4", name="bk4")

                def wqk_memsets():
                    if not k16:
                        for t in (*wq4, *wk4):
                            nc.vector.memset(t[:], 0.0)

                def wqk_build():
                    bpq = dps.tile([RW, 1], f32, tag="d", name="d")
                    nc.tensor.matmul(bpq[:], rep4[:], bq_sb[:], start=True, stop=True)
                    bpk = dps.tile([RW, 1], f32, tag="d", name="d")
                    nc.tensor.matmul(bpk[:], rep4[:], bk_sb[:], start=True, stop=True)
                    nc.vector.tensor_copy(bq4[:], bpq[:])
                    nc.vector.tensor_copy(bk4[:], bpk[:])
                    for ci in range(2):
                        psq = dps.tile([P, D], f32, tag="d", name="d")
                        nc.tensor.transpose(
                            psq[:], wq_sb[:, ci * P : (ci + 1) * P], ident[:D, :D]
                        )
                        psk = dps.tile([P, D], f32, tag="d", name="d")
                        nc.tensor.transpose(
                            psk[:], wk_sb[:, ci * P : (ci + 1) * P], ident[:D, :D]
                        )
                        for r in range(NB):
                            nc.vector.tensor_copy(
                                wq4[ci][:, 32 * r : 32 * r + D], psq[:]
                            )
                            nc.vector.tensor_copy(
                                wk4[ci][:, 32 * r : 32 * r + D], psk[:]
                            )

                def kproj(j):  # key strip j -> k_rep[0:RW, jsl]
                    sl = slice(j * QS, (j + 1) * QS)
                    kp = dps.tile([P, QS], f32, tag="d", name="d")
                    for ci in range(2):
                        nc.tensor.matmul(
                            kp[0:RW, :],
                            wk4[ci][:],
                            xb[ci][:, sl],
                            start=(ci == 0),
                            stop=(ci == 1),
                        )
                    nc.vector.tensor_scalar_add(
                        k_rep[0:RW, sl], kp[0:RW, :], bk4[:]
                    )

                def qproj(s):  # query strip s -> q_rep[s % 2]
                    qp = dps.tile([P, QS], f32, tag="d", name="d")
                    sl = slice(s * QS, (s + 1) * QS)
                    for ci in range(2):
                        nc.tensor.matmul(
                            qp[0:RW, :],
                            wq4[ci][:],
                            xb[ci][:, sl],
                            start=(ci == 0),
                            stop=(ci == 1),
                        )
                    nc.vector.tensor_scalar_add(
                        q_rep[s % 2][0:RW, :], qp[0:RW, :], bq4[:]
                    )

                # ---- W2 = Wo@Wv prep (bf16 on the PE) + epilogue constants
                def w2_prep():
                    wvb, bvb = [], []
                    for ei in range(2):
                        t = wpool.tile([P, C], bf16, tag=f"wvb{ei}", name=f"wvb{ei}")
                        nc.vector.tensor_copy(t[:], wv_sb[:, ei, :])
                        wvb.append(t)
                        t = wpool.tile([P, 1], bf16, tag=f"bvb{ei}", name=f"bvb{ei}")
                        nc.vector.tensor_copy(t[:], bv_sb[:, ei : ei + 1])
                        bvb.append(t)
                    woT = [
                        wpool.tile([P, C], bf16, tag=f"woT{ei}", name=f"woT{ei}")
                        for ei in range(2)
                    ]
                    for ci in range(2):
                        for ei in range(2):
                            ps = dps.tile([P, P], f32, tag="d", name="d")
                            nc.tensor.transpose(
                                ps[:], wo_sb[:, ci, ei * P : (ei + 1) * P], ident[:]
                            )
                            nc.vector.tensor_copy(
                                woT[ei][:, ci * P : (ci + 1) * P], ps[:]
                            )
                    w2T = [
                        wpool.tile([P, C], bf16, tag=f"w2T{ci}", name=f"w2T{ci}")
                        for ci in range(2)
                    ]
                    for ci in range(2):
                        ps = uaps.tile([P, C], f32, tag="ua", name="ua")
                        for ei in range(2):
                            nc.tensor.matmul(
                                ps[:],
                                wvb[ei][:, ci * P : (ci + 1) * P],
                                woT[ei][:],
                                start=(ei == 0),
                                stop=(ei == 1),
                            )
                        nc.vector.tensor_copy(w2T[ci][:], ps[:])
                    return woT, w2T, bvb

                def epi_prep(woT, bvb):
                    # gamma clipped to [0, 1] in place (already broadcast)
                    nc.vector.tensor_scalar(
                        g_rep[:], g_rep[:], 1.0, 0.0, ALU.min, ALU.max
                    )
                    # ub = Wo@bv folds into the epilogue constant:
                    # (ua + ub*den)/den = ua/den + ub, so gbo = g*(bo + ub).
                    gbo = []
                    for fi in range(2):
                        ps = dps.tile([P, 1], f32, tag="d", name="d")
                        for ei in range(2):
                            nc.tensor.matmul(
                                ps[:],
                                woT[ei][:, fi * P : (fi + 1) * P],
                                bvb[ei][:],
                                start=(ei == 0),
                                stop=(ei == 1),
                            )
                        nc.vector.tensor_tensor(
                            ps[:], ps[:], bo_sb[:, fi : fi + 1], ALU.add
                        )
                        t = wpool.tile([P, 1], f32, tag=f"gbo{fi}", name=f"gbo{fi}")
                        nc.vector.tensor_mul(t[:], ps[:], g_rep[:])
                        gbo.append(t)
                    return gbo

                # U^T[k, f] = (W2 @ x)^T chunk, quantized e5m2 for DoubleRow
                ut = big.tile([P, NKC, C], f8e5, tag="ut", name="ut")

                def ut_chunk(kc, w2T):
                    ups = uaps.tile([P, C], f32, tag="ua", name="ua")
                    for ci in range(2):
                        nc.tensor.matmul(
                            ups[:],
                            xb[ci][:, kc * P : (kc + 1) * P],
                            w2T[ci][:],
                            start=(ci == 0),
                            stop=(ci == 1),
                        )
                    nc.vector.tensor_copy(ut[:, kc, :], ups[:])

                # ---------- attention pipeline
                def emit_qk(s, g):
                    qk = qkps.tile([P, KGRP * QS], f32, tag="qk", name="qk")
                    for j in range(KGRP):
                        kc = KGRP * g + j
                        roff = 32 * (kc % NB)
                        nc.tensor.matmul(
                            qk[:, j * QS : (j + 1) * QS],
                            k_rep[roff : roff + KW, kc * P : (kc + 1) * P],
                            q_rep[s % 2][roff : roff + KW, :],
                            start=True,
                            stop=True,
                            tile_position=(roff, 0) if NB > 1 else None,
                        )
                    return qk

                def emit_exp(qk, exc, g, split=True):
                    if g in dve_groups:
                        nc.vector.tensor_scalar(
                            exc[:].bitcast(dt.uint8),
                            qk[:],
                            PLS,
                            PLB,
                            ALU.mult,
                            ALU.add,
                        )
                    elif dve_cols and split:
                        # column-split: ACT and DVE exp the same qk tile
                        # concurrently (PWL 2^y bit trick on the DVE side)
                        a = KGRP * QS - dve_cols
                        nc.vector.tensor_scalar(
                            exc[:, a:].bitcast(dt.uint8),
                            qk[:, a:],
                            PLS,
                            PLB,
                            ALU.mult,
                            ALU.add,
                        )
                        nc.scalar.activation(
                            exc[:, :a], qk[:, :a], AF.Exp, scale=SCALE
                        )
                    else:
                        nc.scalar.activation(exc[:], qk[:], AF.Exp, scale=SCALE)

                # AV/den for strip sm1 (one strip late), group g, reading the
                # SBUF exp cache. ua/den accumulate over all 16 groups.
                def _den(den, exc_tiles, gg, n1):
                    r2g = exc_tiles[gg][:].rearrange("p (a q) -> p a q", a=2)
                    nc.tensor.matmul(
                        den[:],
                        ones_82[:],
                        r2g,
                        start=(gg == 0),
                        stop=(gg == n1),
                        perf_mode=PM.DoubleRow,
                    )

                def emit_av(sm1, g, ua, den, exc_tiles):
                    kc0 = KGRP * g
                    r2 = exc_tiles[g][:].rearrange("p (a q) -> p a q", a=2)
                    for fi in range(2):
                        nc.tensor.matmul(
                            ua[fi][:],
                            ut[:, kc0 : kc0 + 2, fi * P : (fi + 1) * P],
                            r2,
                            start=(g == 0),
                            stop=(g == NGRP - 1),
                            perf_mode=PM.DoubleRow,
                        )
                    # den batched in pairs unless deferred to the strip burst
                    if not den_b and g % 2 == 1:
                        _den(den, exc_tiles, g - 1, NGRP - 1)
                        _den(den, exc_tiles, g, NGRP - 1)

                def den_burst(den, exc_tiles):
                    # all 16 den matmuls consecutive: one `ones` LDWEIGHTS
                    # for the whole strip
                    if den_b:
                        for gg in range(NGRP):
                            _den(den, exc_tiles, gg, NGRP - 1)

                def mk_epilogue(sm1, ua, den, gbo):
                    # split into single DVE ops so they interleave between
                    # the next strip's exp DVE-halves (no head-of-line block)
                    sl = slice(sm1 * QS, (sm1 + 1) * QS)
                    srep = finp.tile([P, QS], f32, tag="srep", name="srep")
                    yts = [
                        finp.tile([P, QS], f32, tag="yt", name="yt")
                        for _ in range(2)
                    ]
                    ops = [lambda: nc.vector.reciprocal(srep[:], den[:])]
                    for fi in range(2):
                        yt = yts[fi]
                        ops.append(
                            lambda fi=fi, yt=yt: nc.vector.tensor_mul(
                                yt[:], ua[fi][:], srep[:]
                            )
                        )
                        ops.append(
                            lambda fi=fi, yt=yt: nc.vector.tensor_scalar(
                                yt[:], yt[:], g_rep[:], gbo[fi][:],
                                ALU.mult, ALU.add,
                            )
                        )

                        def _fin(fi=fi, yt=yt):
                            nc.vector.tensor_add(yt[:], yt[:], xs[fi][:, sl])
                            nc.gpsimd.dma_start(y3[fi, :, sl], yt[:])

                        ops.append(_fin)
                    return ops

                # ---------- emission schedule
                wqk_memsets()
                xb_copy(0)
                xb_copy(1)
                wqk_build()
                kproj(0)
                qproj(0)
                kproj(1)
                xb_copy(2)

                exc_hist = {}   # strip -> list of 16 exp tiles
                av_state = {}   # sm1 -> (ua, den)
                woT = w2T = bvb = gbo = None

                qk = emit_qk(0, 0)
                pend = []
                for s in range(NSTRIP):
                    exc_hist[s] = []
                    last = s == NSTRIP - 1
                    for g in range(NGRP):
                        exc = expp.tile(
                            [P, KGRP * QS], f8e5, tag="exp", name="exp"
                        )
                        emit_exp(qk, exc, g, split=s > 0)
                        exc_hist[s].append(exc)
                        if g + 1 < NGRP:
                            qk = emit_qk(s, g + 1)
                        elif not last:
                            qk = emit_qk(s + 1, 0)
                        if pend:
                            pend.pop(0)()
                        if s == 0:
                            # production rides in strip 0's ACT shadow
                            if g == 0:
                                woT, w2T, bvb = w2_prep()
                                for kc in range(2):
                                    ut_chunk(kc, w2T)
                            elif g % 2 == 0 and g // 2 + 1 < NSTRIP:
                                kproj(g // 2 + 1)
                                if g // 2 + 2 < NSTRIP:
                                    xb_copy(g // 2 + 2)
                            elif g % 2 == 1:
                                for kc in range(2 * g, 2 * g + 4):
                                    if 2 <= kc < NKC:
                                        ut_chunk(kc, w2T)
                            if g == 3:
                                gbo = epi_prep(woT, bvb)
                            if g == NGRP - 3:
                                qproj(1)
                        else:
                            sm1 = s - 1
                            if g == 0:
                                av_state[sm1] = (
                                    [
                                        uaps.tile([P, QS], f32, tag="ua", name="ua")
                                        for _ in range(2)
                                    ],
                                    dps.tile([P, QS], f32, tag="d", name="d"),
                                )
                            ua, den = av_state[sm1]
                            emit_av(sm1, g, ua, den, exc_hist[sm1])
                            if g == NGRP - 1:
                                den_burst(den, exc_hist[sm1])
                                pend.extend(mk_epilogue(sm1, ua, den, gbo))
                                del exc_hist[sm1], av_state[sm1]
                            if g == 2 and not last:
                                qproj(s + 1)

                # tail: AV for the last strip
                sm1 = NSTRIP - 1
                ua = [uaps.tile([P, QS], f32, tag="ua", name="ua") for _ in range(2)]
                den = dps.tile([P, QS], f32, tag="d", name="d")
                for g in range(NGRP):
                    emit_av(sm1, g, ua, den, exc_hist[sm1])
                    if pend:
                        pend.pop(0)()
                den_burst(den, exc_hist[sm1])
                while pend:
                    pend.pop(0)()
                for op in mk_epilogue(sm1, ua, den, gbo):
                    op()

            if loop_repeat:
                with tc.For_i(0, loop_repeat):
                    _emit_rep()
            else:
                for _ in range(repeat):
                    _emit_rep()

    nc.compile()
    return nc


def _get_nc(**kw):
    key = tuple(sorted(kw.items()))
    if key not in _cache:
        _cache[key] = _build(**kw)
    return _cache[key]


def _in_maps(inputs):
    names = ["Wq", "bq", "Wk", "bk", "Wv", "bv", "Wo", "bo", "gamma"]
    base = {
        n: np.ascontiguousarray(np.asarray(inputs[n], dtype=np.float32))
        for n in names
    }
    x = np.ascontiguousarray(np.asarray(inputs["x"], dtype=np.float32))
    assert x.shape == (B, C, 64, 64), x.shape
    maps = []
    for b in range(B):
        m = dict(base)
        m["x"] = np.ascontiguousarray(x[b].reshape(C, HW))
        maps.append(m)
    return maps


def _run(inputs, trace=False, build_kw=None, **kw):
    from concourse.bass_utils import run_bass_kernel_spmd

    nc = _get_nc(**(build_kw or {}))
    res = run_bass_kernel_spmd(
        nc, _in_maps(inputs), core_ids=list(range(B)), trace=trace, **kw
    )
    y = np.stack([r["y"] for r in res.results]).reshape(B, C, 64, 64)
    return np.ascontiguousarray(y.astype(np.float32)), res


def kernel(**inputs):
    y, _ = _run(inputs)
    return y


# revision 26
# speedup vs baseline: 1.0272x; 1.0272x over previous
"""Trainium2 Bass kernel for nn_CAA_Stable (stable-diffusion style spatial
self-attention over 64x64 feature maps), v2: exp-cache software pipeline.

Reference computation per batch b (C=256 channels, N=64*64=4096 positions):
    q = scale*(Wq@x + bq)  [D=16, N]   (scale folded into the exp)
    k = Wk@x + bk          [D, N]
    logits[n,m] = q[:,n].k[:,m];  w = softmax(logits, axis=m)
    y = gamma_clipped * (Wo @ ((Wv@x+bv) @ w^T) + bo) + x

v2 design (vs v1's serial phase-0 + same-strip AV at ~255us):
  * One-strip-lagged AV: strip s's QK+exp stream runs while the AV/den
    DoubleRow matmuls for strip s-1 consume exp tiles CACHED IN SBUF (expp
    pool, 32 bufs = 2 strips of lag). The PE never waits on the ACT, and --
    the real point -- the ua/den PSUM banks are idle during strip 0, so all
    production work (U^T chunks, k projection) pipelines into strip 0
    instead of running as a ~60us serial phase 0.
  * q/k projections write their NB row-band replicas directly from the PE:
    stationary wq4/wk4 = [wqT 0 wqT 0 ...] (zero-gapped band replication),
    so v1's 48 SBUF->SBUF replication DMAs (~35us of HWDGE) are gone. The
    QK matmuls contract over just the D=16 band rows (k16=1), so no
    zero-padding of band gaps is needed at all.
  * Every dma_start costs ~630ns serialized on the shared HWDGE, so DMAs
    are batched (x in 2 chunks/queue, Wv/Wo/bv/bo pair-loaded) and y writes
    go through the Pool-engine SWDGE. Biases are band-replicated by a tiny
    PE matmul against a constant selector instead of 8 separate DMAs;
    gamma broadcasts straight from DRAM.
  * dve_exp of the 16 exp groups per strip run on the DVE as the
    piecewise-linear 2^y bit trick (byte = logit*s*4*log2(e)+60 == e5m2 of
    exp(s*logit)), balancing the ACT (the v1 bottleneck at ~134us busy)
    against the otherwise underused DVE. The softmax scale rides the exp
    (ACT activation scale operand / the PWL multiplier), not the weights.
  * W2 = Wo@Wv precomputed on the PE; U = W2@x replaces the whole v
    projection; Wo@bv and clipped gamma fold into the epilogue constant.
  * softmax exp -> fp8 e5m2 (zero subnormals/saturation for this logit
    range), AV + ones-matmul denominator in e5m2 DoubleRow (256-key
    contraction per pass).
PSUM: qk 2x[128,1024] double-buffered (4 banks) + ua 2x[128,512] (2, ut
production psums in strip 0) + den/proj scratch [128,512] x2 (2).

Timing: per-rep time = slope between loop_repeat=64 and 256 hardware-loop
builds, device-resident serialized dispatch, median (see bench_slope.py);
host dispatch overhead drifts, single-dispatch measurements are noise.

Sharding: pure data-parallel over batch, one image per NeuronCore, no
collectives. kernel() takes FULL inputs, returns the FULL output.
"""

import numpy as np

B, C, HW, D = 8, 256, 4096, 16
P = 128
QS = 512              # q-strip width (one PSUM bank)
NSTRIP = HW // QS     # 8
NKC = HW // P         # 32 key chunks of 128
KGRP = 2              # key chunks per group (one qk tile)
NGRP = NKC // KGRP    # 16
SCALE = float(D) ** -0.5

_cache = {}

DEFAULTS = dict(dve_exp=4)


def _build(
    dve_exp=None,
    dve_cols=None,
    dve_off=0,
    den_b=1,
    repeat=1,
    loop_repeat=0,
    k16=1,
    repl_pack=1,
    dbg=0,
):
    import ml_dtypes
    import concourse.bacc as bacc
    import concourse.mybir as mybir
    import concourse.tile as tile

    if dve_exp is None:
        dve_exp = DEFAULTS["dve_exp"]
    if dve_cols is None:
        dve_cols = DEFAULTS.get("dve_cols", 0)

    dt = mybir.dt
    AF = mybir.ActivationFunctionType
    ALU = mybir.AluOpType
    f32, bf16, f8e5 = dt.float32, dt.bfloat16, dt.float8e5
    PM = mybir.MatmulPerfMode

    nc = bacc.Bacc("TRN2", target_bir_lowering=False, debug=False, num_devices=B)

    x_d = nc.dram_tensor("x", [C, HW], f32, kind="ExternalInput")
    wq_d = nc.dram_tensor("Wq", [D, C], f32, kind="ExternalInput")
    bq_d = nc.dram_tensor("bq", [D], f32, kind="ExternalInput")
    wk_d = nc.dram_tensor("Wk", [D, C], f32, kind="ExternalInput")
    bk_d = nc.dram_tensor("bk", [D], f32, kind="ExternalInput")
    wv_d = nc.dram_tensor("Wv", [C, C], f32, kind="ExternalInput")
    bv_d = nc.dram_tensor("bv", [C], f32, kind="ExternalInput")
    wo_d = nc.dram_tensor("Wo", [C, C], f32, kind="ExternalInput")
    bo_d = nc.dram_tensor("bo", [C], f32, kind="ExternalInput")
    g_d = nc.dram_tensor("gamma", [1], f32, kind="ExternalInput")
    y_d = nc.dram_tensor("y", [C, HW], f32, kind="ExternalOutput")

    # q/k replicated across NB row bands (32-row spacing for tile_position)
    NB = 4 if repl_pack else 1
    RW = 32 * (NB - 1) + D  # written partition rows: 112 packed, 16 unpacked
    KW = D if k16 else 32   # qk contraction rows per band

    id_d = nc.inline_tensor(np.eye(P, dtype=np.float32), name="ident_c")
    ones82_d = nc.inline_tensor(
        np.ones((P, 2, P), dtype=ml_dtypes.float8_e5m2), name="ones82_c"
    )
    rep4_np = np.zeros((D, RW), dtype=np.float32)
    for r in range(NB):
        rep4_np[np.arange(D), 32 * r + np.arange(D)] = 1.0
    rep4_d = nc.inline_tensor(rep4_np, name="rep4_c")

    x3 = x_d.ap().rearrange("(a p) n -> a p n", p=P)
    y3 = y_d.ap().rearrange("(a p) n -> a p n", p=P)
    wv3 = wv_d.ap().rearrange("(a p) c -> p a c", p=P)
    wo3 = wo_d.ap().rearrange("(a p) c -> p a c", p=P)
    bv2 = bv_d.ap().rearrange("(a p) -> p a", p=P)
    bo2 = bo_d.ap().rearrange("(a p) -> p a", p=P)

    # groups (per strip) whose exp runs on the DVE instead of the ACT:
    # byte = round(logit*scale * 4*log2(e) + 60) == e5m2(2^(logit*scale*log2 e))
    # (carry from the 2-bit mantissa rounds into the exponent field).
    dve_groups = frozenset(
        (g - dve_off) % NGRP
        for g in range(NGRP)
        if ((g + 1) * dve_exp) // NGRP > (g * dve_exp) // NGRP
    )
    PLS = SCALE * 4.0 * 1.4426950408889634
    PLB = 60.0

    with tile.TileContext(nc) as tc:
        with (
            tc.tile_pool(name="const", bufs=1) as constp,
            tc.tile_pool(name="xpool", bufs=1) as xpool,
            tc.tile_pool(name="wpool", bufs=1) as wpool,
            tc.tile_pool(name="big", bufs=1) as big,
            tc.tile_pool(name="expp", bufs=2 * NGRP) as expp,
            tc.tile_pool(name="finp", bufs=2) as finp,
            tc.tile_pool(name="qkps", bufs=2, space="PSUM") as qkps,
            tc.tile_pool(name="uaps", bufs=2, space="PSUM") as uaps,
            tc.tile_pool(name="dps", bufs=2, space="PSUM") as dps,
        ):
            # exp table load with no input deps: reads its own uninit tile
            warm = constp.tile([1, 1], f32, tag="warm", name="warm")
            nc.scalar.activation(warm[:], warm[:], AF.Exp)

            ident = constp.tile([P, P], f32, tag="ident", name="ident")
            ones_82 = constp.tile([P, 2, P], f8e5, tag="ones_82", name="ones_82")
            rep4 = constp.tile([D, RW], f32, tag="rep4", name="rep4")

            # persistent replicated q (2 strip buffers) / k tiles. With k16
            # the qk matmuls only read the D valid rows of each band, so the
            # band gaps and tail rows never need zeroing.
            q_rep = [
                big.tile([P, QS], bf16, tag=f"q_rep{i}", name=f"q_rep{i}")
                for i in range(2)
            ]
            k_rep = big.tile([P, HW], bf16, tag="k_rep", name="k_rep")
            if not k16:
                for t in (*q_rep, k_rep):
                    nc.vector.memset(t[:], 0.0)

            def _emit_rep():
                # ---------- prologue: loads (SP queue, critical first)
                xs = []
                for ci in range(2):
                    t = xpool.tile([P, HW], f32, tag=f"x{ci}", name=f"x{ci}")
                    xs.append(t)
                qq = [nc.sync, nc.gpsimd]
                for ci in range(2):
                    qq[ci].dma_start(xs[ci][:, : 2 * QS], x3[ci][:, : 2 * QS])
                wq_sb = wpool.tile([D, C], f32, tag="wq", name="wq")
                nc.sync.dma_start(wq_sb[:], wq_d.ap())
                wk_sb = wpool.tile([D, C], f32, tag="wk", name="wk")
                nc.sync.dma_start(wk_sb[:], wk_d.ap())
                nc.sync.dma_start(ident[:], id_d.ap())

                with nc.allow_non_contiguous_dma(reason="tiny bias vectors"):
                    bq_sb = wpool.tile([D, 1], f32, tag="bq", name="bq")
                    nc.sync.dma_start(bq_sb[:], bq_d.ap()[:, None])
                    bk_sb = wpool.tile([D, 1], f32, tag="bk", name="bk")
                    nc.sync.dma_start(bk_sb[:], bk_d.ap()[:, None])
                nc.sync.dma_start(rep4[:], rep4_d.ap())
                # x strips 2-7 chunked so no single transfer blocks the small
                # weight loads on the serial DMA device
                for ci in range(2):
                    qq[ci].dma_start(
                        xs[ci][:, 2 * QS : 4 * QS], x3[ci][:, 2 * QS : 4 * QS]
                    )
                wv_sb = wpool.tile([P, 2, C], f32, tag="wv", name="wv")
                nc.sync.dma_start(wv_sb[:], wv3)
                wo_sb = wpool.tile([P, 2, C], f32, tag="wo", name="wo")
                nc.sync.dma_start(wo_sb[:], wo3)
                for ci in range(2):
                    qq[ci].dma_start(
                        xs[ci][:, 4 * QS : 6 * QS], x3[ci][:, 4 * QS : 6 * QS]
                    )
                bv_sb = wpool.tile([P, 2], f32, tag="bv", name="bv")
                nc.sync.dma_start(bv_sb[:], bv2)
                bo_sb = wpool.tile([P, 2], f32, tag="bo", name="bo")
                nc.sync.dma_start(bo_sb[:], bo2)
                g_rep = wpool.tile([P, 1], f32, tag="grep", name="grep")
                nc.sync.dma_start(g_rep[:], g_d.ap()[:, None].to_broadcast((P, 1)))
                for ci in range(2):
                    qq[ci].dma_start(
                        xs[ci][:, 6 * QS :], x3[ci][:, 6 * QS :]
                    )
                nc.sync.dma_start(ones_82[:], ones82_d.ap())

                xb = []
                for ci in range(2):
                    tb = xpool.tile([P, HW], bf16, tag=f"xb{ci}", name=f"xb{ci}")
                    xb.append(tb)

                def xb_copy(j):  # strip j cols -> bf16, both channel tiles
                    sl = slice(j * QS, (j + 1) * QS)
                    for ci in range(2):
                        nc.vector.tensor_copy(xb[ci][:, sl], xs[ci][:, sl])

                # transposed wq/wk replicated at the NB 32-row bands; band-
                # replicated biases via a tiny matmul against the rep4
                # selector (one DMA instead of 8).
                wq4, wk4 = [], []
                for ci in range(2):
                    t = wpool.tile([P, RW], bf16, tag=f"wq4{ci}", name=f"wq4{ci}")
                    wq4.append(t)
                    t = wpool.tile([P, RW], bf16, tag=f"wk4{ci}", name=f"wk4{ci}")
                    wk4.append(t)
                bq4 = wpool.tile([RW, 1], f32, tag="bq4", name="bq4")
                bk4 = wpool.tile([RW, 1], f32, tag="bk4", name="bk4")

                def wqk_memsets():
                    if not k16:
                        for t in (*wq4, *wk4):
                            nc.vector.memset(t[:], 0.0)

                def wqk_build():
                    bpq = dps.tile([RW, 1], f32, tag="d", name="d")
                    nc.tensor.matmul(bpq[:], rep4[:], bq_sb[:], start=True, stop=True)
                    bpk = dps.tile([RW, 1], f32, tag="d", name="d")
                    nc.tensor.matmul(bpk[:], rep4[:], bk_sb[:], start=True, stop=True)
                    nc.vector.tensor_copy(bq4[:], bpq[:])
                    nc.vector.tensor_copy(bk4[:], bpk[:])
                    for ci in range(2):
                        psq = dps.tile([P, D], f32, tag="d", name="d")
                        nc.tensor.transpose(
                            psq[:], wq_sb[:, ci * P : (ci + 1) * P], ident[:D, :D]
                        )
                        psk = dps.tile([P, D], f32, tag="d", name="d")
                        nc.tensor.transpose(
                            psk[:], wk_sb[:, ci * P : (ci + 1) * P], ident[:D, :D]
                        )
                        for r in range(NB):
                            nc.vector.tensor_copy(
                                wq4[ci][:, 32 * r : 32 * r + D], psq[:]
                            )
                            nc.vector.tensor_copy(
                                wk4[ci][:, 32 * r : 32 * r + D], psk[:]
                            )

                def kproj(j):  # key strip j -> k_rep[0:RW, jsl]
                    sl = slice(j * QS, (j + 1) * QS)
                    kp = dps.tile([P, QS], f32, tag="d", name="d")
                    for ci in range(2):
                        nc.tensor.matmul(
                            kp[0:RW, :],
                            wk4[ci][:],
                            xb[ci][:, sl],
                            start=(ci == 0),
                            stop=(ci == 1),
                        )
                    nc.vector.tensor_scalar_add(
                        k_rep[0:RW, sl], kp[0:RW, :], bk4[:]
                    )

                def qproj(s):  # query strip s -> q_rep[s % 2]
                    qp = dps.tile([P, QS], f32, tag="d", name="d")
                    sl = slice(s * QS, (s + 1) * QS)
                    for ci in range(2):
                        nc.tensor.matmul(
                            qp[0:RW, :],
                            wq4[ci][:],
                            xb[ci][:, sl],
                            start=(ci == 0),
                            stop=(ci == 1),
                        )
                    nc.vector.tensor_scalar_add(
                        q_rep[s % 2][0:RW, :], qp[0:RW, :], bq4[:]
                    )

                # ---- W2 = Wo@Wv prep (bf16 on the PE) + epilogue constants
                def w2_prep():
                    wvb, bvb = [], []
                    for ei in range(2):
                        t = wpool.tile([P, C], bf16, tag=f"wvb{ei}", name=f"wvb{ei}")
                        nc.vector.tensor_copy(t[:], wv_sb[:, ei, :])
                        wvb.append(t)
                        t = wpool.tile([P, 1], bf16, tag=f"bvb{ei}", name=f"bvb{ei}")
                        nc.vector.tensor_copy(t[:], bv_sb[:, ei : ei + 1])
                        bvb.append(t)
                    woT = [
                        wpool.tile([P, C], bf16, tag=f"woT{ei}", name=f"woT{ei}")
                        for ei in range(2)
                    ]
                    for ci in range(2):
                        for ei in range(2):
                            ps = dps.tile([P, P], f32, tag="d", name="d")
                            nc.tensor.transpose(
                                ps[:], wo_sb[:, ci, ei * P : (ei + 1) * P], ident[:]
                            )
                            nc.vector.tensor_copy(
                                woT[ei][:, ci * P : (ci + 1) * P], ps[:]
                            )
                    w2T = [
                        wpool.tile([P, C], bf16, tag=f"w2T{ci}", name=f"w2T{ci}")
                        for ci in range(2)
                    ]
                    for ci in range(2):
                        ps = uaps.tile([P, C], f32, tag="ua", name="ua")
                        for ei in range(2):
                            nc.tensor.matmul(
                                ps[:],
                                wvb[ei][:, ci * P : (ci + 1) * P],
                                woT[ei][:],
                                start=(ei == 0),
                                stop=(ei == 1),
                            )
                        nc.vector.tensor_copy(w2T[ci][:], ps[:])
                    return woT, w2T, bvb

                def epi_prep(woT, bvb):
                    # gamma clipped to [0, 1] in place (already broadcast)
                    nc.vector.tensor_scalar(
                        g_rep[:], g_rep[:], 1.0, 0.0, ALU.min, ALU.max
                    )
                    # ub = Wo@bv folds into the epilogue constant:
                    # (ua + ub*den)/den = ua/den + ub, so gbo = g*(bo + ub).
                    gbo = []
                    for fi in range(2):
                        ps = dps.tile([P, 1], f32, tag="d", name="d")
                        for ei in range(2):
                            nc.tensor.matmul(
                                ps[:],
                                woT[ei][:, fi * P : (fi + 1) * P],
                                bvb[ei][:],
                                start=(ei == 0),
                                stop=(ei == 1),
                            )
                        nc.vector.tensor_tensor(
                            ps[:], ps[:], bo_sb[:, fi : fi + 1], ALU.add
                        )
                        t = wpool.tile([P, 1], f32, tag=f"gbo{fi}", name=f"gbo{fi}")
                        nc.vector.tensor_mul(t[:], ps[:], g_rep[:])
                        gbo.append(t)
                    return gbo

                # U^T[k, f] = (W2 @ x)^T chunk, quantized e5m2 for DoubleRow
                ut = big.tile([P, NKC, C], f8e5, tag="ut", name="ut")

                def ut_chunk(kc, w2T):
                    ups = uaps.tile([P, C], f32, tag="ua", name="ua")
                    for ci in range(2):
                        nc.tensor.matmul(
                            ups[:],
                            xb[ci][:, kc * P : (kc + 1) * P],
                            w2T[ci][:],
                            start=(ci == 0),
                            stop=(ci == 1),
                        )
                    nc.vector.tensor_copy(ut[:, kc, :], ups[:])

                # ---------- attention pipeline
                def emit_qk(s, g):
                    qk = qkps.tile([P, KGRP * QS], f32, tag="qk", name="qk")
                    for j in range(KGRP):
                        kc = KGRP * g + j
                        roff = 32 * (kc % NB)
                        nc.tensor.matmul(
                            qk[:, j * QS : (j + 1) * QS],
                            k_rep[roff : roff + KW, kc * P : (kc + 1) * P],
                            q_rep[s % 2][roff : roff + KW, :],
                            start=True,
                            stop=True,
                            tile_position=(roff, 0) if NB > 1 else None,
                        )
                    return qk

                def emit_exp(qk, exc, g, split=True):
                    if g in dve_groups:
                        nc.vector.tensor_scalar(
                            exc[:].bitcast(dt.uint8),
                            qk[:],
                            PLS,
                            PLB,
                            ALU.mult,
                            ALU.add,
                        )
                    elif dve_cols and split:
                        # column-split: ACT and DVE exp the same qk tile
                        # concurrently (PWL 2^y bit trick on the DVE side)
                        a = KGRP * QS - dve_cols
                        nc.vector.tensor_scalar(
                            exc[:, a:].bitcast(dt.uint8),
                            qk[:, a:],
                            PLS,
                            PLB,
                            ALU.mult,
                            ALU.add,
                        )
                        nc.scalar.activation(
                            exc[:, :a], qk[:, :a], AF.Exp, scale=SCALE
                        )
                    else:
                        nc.scalar.activation(exc[:], qk[:], AF.Exp, scale=SCALE)

                # AV/den for strip sm1 (one strip late), group g, reading the
                # SBUF exp cache. ua/den accumulate over all 16 groups.
                def _den(den, exc_tiles, gg, n1):
                    r2g = exc_tiles[gg][:].rearrange("p (a q) -> p a q", a=2)
                    nc.tensor.matmul(
                        den[:],
                        ones_82[:],
                        r2g,
                        start=(gg == 0),
                        stop=(gg == n1),
                        perf_mode=PM.DoubleRow,
                    )

                def emit_av(sm1, g, ua, den, exc_tiles):
                    kc0 = KGRP * g
                    r2 = exc_tiles[g][:].rearrange("p (a q) -> p a q", a=2)
                    for fi in range(2):
                        nc.tensor.matmul(
                            ua[fi][:],
                            ut[:, kc0 : kc0 + 2, fi * P : (fi + 1) * P],
                            r2,
                            start=(g == 0),
                            stop=(g == NGRP - 1),
                            perf_mode=PM.DoubleRow,
                        )
                    # den batched in pairs unless deferred to the strip burst
                    if not den_b and g % 2 == 1:
                        _den(den, exc_tiles, g - 1, NGRP - 1)
                        _den(den, exc_tiles, g, NGRP - 1)

                def den_burst(den, exc_tiles):
                    # all 16 den matmuls consecutive: one `ones` LDWEIGHTS
                    # for the whole strip
                    if den_b:
                        for gg in range(NGRP):
                            _den(den, exc_tiles, gg, NGRP - 1)

                def mk_epilogue(sm1, ua, den, gbo):
                    # split into single DVE ops so they interleave between
                    # the next strip's exp DVE-halves (no head-of-line block)
                    sl = slice(sm1 * QS, (sm1 + 1) * QS)
                    srep = finp.tile([P, QS], f32, tag="srep", name="srep")
                    yts = [
                        finp.tile([P, QS], f32, tag="yt", name="yt")
                        for _ in range(2)
                    ]
                    ops = [lambda: nc.vector.reciprocal(srep[:], den[:])]
                    for fi in range(2):
                        yt = yts[fi]
                        ops.append(
                            lambda fi=fi, yt=yt: nc.vector.tensor_mul(
                                yt[:], ua[fi][:], srep[:]
                            )
                        )
                        ops.append(
                            lambda fi=fi, yt=yt: nc.vector.tensor_scalar(
                                yt[:], yt[:], g_rep[:], gbo[fi][:],
                                ALU.mult, ALU.add,
                            )
                        )

                        def _fin(fi=fi, yt=yt):
                            nc.vector.tensor_add(yt[:], yt[:], xs[fi][:, sl])
                            nc.gpsimd.dma_start(y3[fi, :, sl], yt[:])

                        ops.append(_fin)
                    return ops

                # ---------- emission schedule
                wqk_memsets()
                xb_copy(0)
                xb_copy(1)
                wqk_build()
                kproj(0)
                qproj(0)
                kproj(1)
                xb_copy(2)

                exc_hist = {}   # strip -> list of 16 exp tiles
                av_state = {}   # sm1 -> (ua, den)
                woT = w2T = bvb = gbo = None

                qk = emit_qk(0, 0)
                pend = []
                for s in range(NSTRIP):
                    exc_hist[s] = []
                    last = s == NSTRIP - 1
                    for g in range(NGRP):
                        exc = expp.tile(
                            [P, KGRP * QS], f8e5, tag="exp", name="exp"
                        )
                        emit_exp(qk, exc, g, split=s > 0)
                        exc_hist[s].append(exc)
                        if g + 1 < NGRP:
                            qk = emit_qk(s, g + 1)
                        elif not last:
                            qk = emit_qk(s + 1, 0)
                        if pend:
                            pend.pop(0)()
                        if s == 0:
                            # production rides in strip 0's ACT shadow
                            if g == 0:
                                woT, w2T, bvb = w2_prep()
                                for kc in range(2):
                                    ut_chunk(kc, w2T)
                            elif g % 2 == 0 and g // 2 + 1 < NSTRIP:
                                kproj(g // 2 + 1)
                                if g // 2 + 2 < NSTRIP:
                                    xb_copy(g // 2 + 2)
                            elif g % 2 == 1:
                                for kc in range(2 * g, 2 * g + 4):
                                    if 2 <= kc < NKC:
                                        ut_chunk(kc, w2T)
                            if g == 3:
                                gbo = epi_prep(woT, bvb)
                            if g == NGRP - 3:
                                qproj(1)
                        else:
                            sm1 = s - 1
                            if g == 0:
                                av_state[sm1] = (
                                    [
                                        uaps.tile([P, QS], f32, tag="ua", name="ua")
                                        for _ in range(2)
                                    ],
                                    dps.tile([P, QS], f32, tag="d", name="d"),
                                )
                            ua, den = av_state[sm1]
                            emit_av(sm1, g, ua, den, exc_hist[sm1])
                            if g == NGRP - 1:
                                den_burst(den, exc_hist[sm1])
                                pend.extend(mk_epilogue(sm1, ua, den, gbo))
                                del exc_hist[sm1], av_state[sm1]
                            if g == 2 and not last:
                                qproj(s + 1)

                # tail: AV for the last strip
                sm1 = NSTRIP - 1
                ua = [uaps.tile([P, QS], f32, tag="ua", name="ua") for _ in range(2)]
                den = dps.tile([P, QS], f32, tag="d", name="d")
                for g in range(NGRP):
                    emit_av(sm1, g, ua, den, exc_hist[sm1])
                    if pend:
                        pend.pop(0)()
                den_burst(den, exc_hist[sm1])
                while pend:
                    pend.pop(0)()
                for op in mk_epilogue(sm1, ua, den, gbo):
                    op()

            if loop_repeat:
                with tc.For_i(0, loop_repeat):
                    _emit_rep()
            else:
                for _ in range(repeat):
                    _emit_rep()

    nc.compile()
    return nc


def _get_nc(**kw):
    key = tuple(sorted(kw.items()))
    if key not in _cache:
        _cache[key] = _build(**kw)
    return _cache[key]


def _in_maps(inputs):
    names = ["Wq", "bq", "Wk", "bk", "Wv", "bv", "Wo", "bo", "gamma"]
    base = {
        n: np.ascontiguousarray(np.asarray(inputs[n], dtype=np.float32))
        for n in names
    }
    x = np.ascontiguousarray(np.asarray(inputs["x"], dtype=np.float32))
    assert x.shape == (B, C, 64, 64), x.shape
    maps = []
    for b in range(B):
        m = dict(base)
        m["x"] = np.ascontiguousarray(x[b].reshape(C, HW))
        maps.append(m)
    return maps


def _run(inputs, trace=False, build_kw=None, **kw):
    from concourse.bass_utils import run_bass_kernel_spmd

    nc = _get_nc(**(build_kw or {}))
    res = run_bass_kernel_spmd(
        nc, _in_maps(inputs), core_ids=list(range(B)), trace=trace, **kw
    )
    y = np.stack([r["y"] for r in res.results]).reshape(B, C, 64, 64)
    return np.ascontiguousarray(y.astype(np.float32)), res


def kernel(**inputs):
    y, _ = _run(inputs)
    return y
